# revision 64
# baseline (speedup 1.0000x reference)
"""Causal self-attention, head-tensor-parallel across 8 TRN2 NeuronCores.

Problem: x[2,2048,1024] -> qkv = x@W_attn+b_attn -> 16-head causal attention
(head dim 64) -> y@W_proj+b_proj.

Sharding: heads are tensor-parallel. Core c owns heads 2c and 2c+1:
  - W_attn column slices for its q/k/v features (384 cols), W_proj row slice
    (128 rows). Every core reads all of x (transposed+bf16 on host).
  - Each core emits a full [4096,1024] fp16 partial of the output projection;
    the host sums the 8 partials and adds b_proj.

On-core dataflow (all matmuls bf16 in / fp32 PSUM accum):
  1. qkv^T[384,4096] = W_slice^T @ x^T (features on partitions). Chunk 0
     streams in fine-grained pieces (first piece at the head of the sync
     queue) and runs as two half-token passes so the PE starts ~11us in.
  2. V_aug = PE-transpose of v^T + a trailing ones column per head: row
     sums of P fall out of the AV matmul as the softmax denominator.
  3. S^T[k,q] = k^T.T @ q^T per head, causally block-skipped over a packed
     ragged column space; the two heads run as PE row-group pairs (K=64 at
     partitions 0-63 / 64-127) which co-execute -> full PE rate. exp via
     ScalarE (scale=1/8), bf16 out = P^T. Diagonal blocks are masked by a
     DVE multiply with a DRAM-loaded triu constant (NOT gpsimd
     affine_select: mixing gpsimd op families forces ~6us ucode library
     swaps, so gpsimd runs ONLY partition_broadcast + DMA issues).
  4. AV in y^T orientation per 512-token chunk (two 256/128-token chunks at
     the very end to shorten the drain): V_aug stationary, P^T moving,
     accumulated over k-tiles in PSUM. Unpaired (row-group-paired AV loses
     more to PSUM-bank WAR serialization and extra evict work than it saves
     in PE time -- measured, twice).
  5. Normalize without transposes: AV PSUM -> SBUF stage (DVE or ScalarE),
     reciprocal_approx_fast on the denominator row (p0 via a tiny SB->SB
     DMA lane shift; both custom-DVE ops and partition_broadcast are
     partition-0-only), one partition_broadcast, then DVE muls; head A
     lands straight in its yT tile, head B via an SB->SB DMA lane shift to
     rows 64-127. One yT tile per chunk so projection tails only wait on
     their own chunk's chain.
  6. out[tok,1024] = yT.T @ W_proj_slice per 128-token tile, fp16 evict
     split across DVE+ScalarE, store DMAs fanned over sync/scalar/gpsimd
     queues at the drain. Everything is software-pipelined one stage so the
     PE stream never parks on a normalize chain.
  7. S(b1)'s windows are interleaved INTO the b0 AV loop, grouped by the
     last b0 chunk whose pt reads each window's exp must WAR-wait: exp(b1)
     streams on ScalarE concurrently with b0's chains/tails instead of
     serializing after all of AV(b0). Each causal mask fires with the
     window that CONTAINS its diagonal block (windows emit out of order).
"""

import numpy as np
import ml_dtypes

import concourse.bacc as bacc
import concourse.bass as bass
import concourse.mybir as mybir
import concourse.tile as tile
from concourse.bass_utils import run_bass_kernel_spmd
from concourse.masks import make_identity

BF16 = mybir.dt.bfloat16
FP16 = mybir.dt.float16
FP32 = mybir.dt.float32

B, T, C, H = 2, 2048, 1024, 16
D = C // H            # 64
N_CORES = 8
HPC = H // N_CORES    # heads per core = 2
TOK = B * T           # 4096
P = 128               # partitions / tile edge
KT = T // P           # 16 k/q tiles per batch element
NQ = 1024             # S^T / exp chunk width for b0 (2 PSUM banks)
XC = 1024             # x^T token chunk for streaming
QQ = 512              # AV accumulator width (1 PSUM bank per accumulator)

bf16 = ml_dtypes.bfloat16


def _pt_offsets():
    """Column offsets of each k-tile's ragged [k, q] strip in the P^T store."""
    offs, total = [], 0
    for t in range(KT):
        offs.append(total)
        total += T - P * t
    return offs, total


PT_OFF, PT_COLS = _pt_offsets()  # PT_COLS = 17408


def build_nc():
    # Bacc (not raw Bass): its lowering legalizes TRN2's one-wait-per-
    # instruction constraint by splitting multi-waits into EventSemaphores.
    nc = bacc.Bacc("TRN2", target_bir_lowering=False, debug=False)

    # x^T pre-packed on host into contiguous per-chunk DMA layout:
    # [chunk, p, ktile, tok] with element = x[chunk*XC+tok, ktile*128+p]
    xT = nc.dram_tensor(
        "xT", [TOK // XC, P, C // P, XC], BF16, kind="ExternalInput"
    ).ap()
    w_qkv = nc.dram_tensor("w_qkv", [C, 3 * P], BF16, kind="ExternalInput").ap()
    b_qkv = nc.dram_tensor("b_qkv", [P, 3], FP32, kind="ExternalInput").ap()
    w_p = nc.dram_tensor("w_p", [P, C], BF16, kind="ExternalInput").ap()
    tri = nc.dram_tensor("tri", [P, P], BF16, kind="ExternalInput").ap()
    out_p = nc.dram_tensor("out_p", [TOK, C], FP16, kind="ExternalOutput").ap()

    with TileOwner(nc) as tc:
        _emit(nc, tc, xT, w_qkv, b_qkv, w_p, tri, out_p)
    nc.compile()
    return nc


class TileOwner:
    """Thin wrapper so build_nc reads top-down; just a TileContext."""

    def __init__(self, nc):
        self._tc = tile.TileContext(nc)

    def __enter__(self):
        return self._tc.__enter__()

    def __exit__(self, *a):
        return self._tc.__exit__(*a)


def _emit(nc, tc, xT, w_qkv, b_qkv, w_p, tri, out_p):
    from contextlib import ExitStack

    ctx = ExitStack()
    with ctx:
        consts = ctx.enter_context(tc.tile_pool(name="consts", bufs=1))
        persist = ctx.enter_context(tc.tile_pool(name="persist", bufs=1))

        # ---- constants ----
        # DMA queue plan: w_qkv k-tile slices + bias + later chunks ride the
        # sync HWDGE queue; x chunk-0 pieces stream alone on the gpsimd
        # queue so nothing queues ahead of the piece the first matmul needs.
        # bias is host-prepacked to [128,3] (the rearrange gather emitted
        # 384 4-byte descriptors that clogged the queue for ~10us).
        w_qkv_sb = consts.tile([P, C // P, 3 * P], BF16)  # [p, ktile, feat]
        bias_sb = consts.tile([P, 3], FP32)  # col m: bias of feature m*128+p
        w_p_sb = consts.tile([P, C], BF16)
        tri_sb = consts.tile([P, P], BF16)  # triu ones: causal keep-mask
        ident = consts.tile([P, P], BF16)

        # ---- persistent activations ----
        qT = persist.tile([P, TOK], BF16)   # rows: head A dims 0-63, head B 64-127
        kTt = persist.tile([P, TOK], BF16)
        vT = persist.tile([P, TOK], BF16)
        qkvT = [qT, kTt, vT]
        # V augmented with ones column, per (b, head): [k-in-tile, ktile, D+1]
        # head A: [v(64), ones]; head B: [ones, v(64)] (so head B's AV output
        # can sit at PSUM partitions 63-127: denom 63, y dims 64-127).
        v_aug = [
            persist.tile([P, KT, HPC, D + 1], BF16, name=f"v_aug_{b}")
            for b in range(B)
        ]
        # normalized y^T, feat on partitions -- one tile per norm-chain
        # chunk so projection tails only wait on THEIR chunk's chain.
        # b1's last 512 tokens run as two 256-wide chains to halve the
        # end-of-kernel drain latency.
        CHUNKS = [(b, t0, t1) for b in range(B) for (t0, t1) in
                  ([(i * QQ, (i + 1) * QQ) for i in range(T // QQ)] if b == 0
                   else [(0, QQ), (QQ, 2 * QQ), (2 * QQ, 3 * QQ),
                         (3 * QQ, 3 * QQ + 256), (3 * QQ + 256, 3 * QQ + 384),
                         (3 * QQ + 384, T)])]
        yT_sb = [
            persist.tile([P, t1 - t0], BF16, name=f"yT_{i}")
            for i, (b, t0, t1) in enumerate(CHUNKS)
        ]
        # gq (128-token projection tile) -> (chunk index, col offset)
        GQ_MAP = {}
        for ci, (b, t0, t1) in enumerate(CHUNKS):
            for qt in range((t1 - t0) // P):
                GQ_MAP[b * KT + t0 // P + qt] = (ci, qt * P)
        # ragged P^T store, one per head (reused across b; serializes b0/b1)
        pt_sb = [persist.tile([P, PT_COLS], BF16, name=f"pt_{h}") for h in range(HPC)]

        # ---- pipeline ----
        # SBUF-side tail pools
        osb = ctx.enter_context(tc.tile_pool(name="o_sb", bufs=4))
        stage_pool = ctx.enter_context(tc.tile_pool(name="y_stage", bufs=3))
        bc_pool = ctx.enter_context(tc.tile_pool(name="bcast", bufs=2))
        tmpb_pool = ctx.enter_context(tc.tile_pool(name="ytmp_b", bufs=4))
        avsb_pool = ctx.enter_context(tc.tile_pool(name="av_rg1", bufs=2))
        rcp_pool = ctx.enter_context(tc.tile_pool(name="rcp_row", bufs=2))
        bsrc_pool = ctx.enter_context(tc.tile_pool(name="bsrc", bufs=2))
        xp = tc.alloc_tile_pool(name="xT_pool", bufs=2)
        # chunk 0 runs k-tile-outer with all 6 accumulators live so the PE
        # starts as soon as the first (w, x) k-slice pair lands; its pool is
        # released before the steady-state pools go down.
        qps0 = tc.alloc_tile_pool(name="qkv0_ps", bufs=1, space="PSUM")

        def _qkv_evict(out, ps, mi, j):
            # bias-add eviction on DVE (ScalarE is pacing exp(b0) then).
            nc.vector.tensor_scalar_add(
                out=out, in0=ps, scalar1=bias_sb[:, mi : mi + 1]
            )

        def emit_qkv0():
            # x chunk 0 in 2-ktile half-token pieces on the gpsimd queue,
            # w slices on sync in parallel; QKV0 runs as two half-token
            # passes so the first matmuls need only a 128KB piece.
            x_sb = xp.tile([P, C // P, XC], BF16, name="x_sb")
            # the first x piece gates the first matmul: it rides at the
            # HEAD of the sync queue (sync's preamble clears ~1us before
            # gpsimd's); w kt0 follows it, everything else as usual.
            nc.sync.dma_start(out=x_sb[:, 0:1, 0:512], in_=xT[0][:, 0:1, 0:512])
            nc.sync.dma_start(out=w_qkv_sb[:, 0, :], in_=w_qkv[0:P, :])
            nc.gpsimd.dma_start(out=x_sb[:, 1:2, 0:512], in_=xT[0][:, 1:2, 0:512])
            nc.sync.dma_start(out=w_qkv_sb[:, 1, :], in_=w_qkv[P : 2 * P, :])
            for g in range(1, C // P // 2):
                for kt in (2 * g, 2 * g + 1):
                    nc.sync.dma_start(
                        out=w_qkv_sb[:, kt, :], in_=w_qkv[kt * P : (kt + 1) * P, :]
                    )
                nc.gpsimd.dma_start(
                    out=x_sb[:, 2 * g : 2 * g + 2, 0:512],
                    in_=xT[0][:, 2 * g : 2 * g + 2, 0:512],
                )
            for g in range(C // P // 2):
                nc.gpsimd.dma_start(
                    out=x_sb[:, 2 * g : 2 * g + 2, 512:XC],
                    in_=xT[0][:, 2 * g : 2 * g + 2, 512:XC],
                )
            nc.sync.dma_start(out=bias_sb, in_=b_qkv)
            nc.sync.dma_start(out=tri_sb, in_=tri)
            make_identity(nc, ident)
            # warm the gpsimd partition_broadcast microcode library during
            # the startup DMA wait: the first pbcast otherwise pays a ~6us
            # library swap right inside b0's first normalize chain.
            warm = bsrc_pool.tile([D, 8], FP32, name="pb_warm")
            nc.vector.memset(warm[0:1, :], 1.0)
            nc.gpsimd.partition_broadcast(warm, warm[0:1, :])
            accs = [qps0.tile([P, 512], FP32, name=f"q0_acc{i}") for i in range(6)]
            for j in range(2):
                for kt in range(C // P):
                    for mi in range(3):
                        nc.tensor.matmul(
                            accs[mi * 2 + j],
                            w_qkv_sb[:, kt, mi * P : (mi + 1) * P],
                            x_sb[:, kt, j * 512 : (j + 1) * 512],
                            start=(kt == 0),
                            stop=(kt == C // P - 1),
                        )
            for mi in range(3):
                for j in range(2):
                    _qkv_evict(
                        qkvT[mi][:, j * 512 : (j + 1) * 512],
                        accs[mi * 2 + j], mi, j,
                    )

        def emit_qkv(nch, q="sync"):
            x_sb = xp.tile([P, C // P, XC], BF16, name="x_sb")
            getattr(nc, q).dma_start(out=x_sb[:, :, 0:512], in_=xT[nch][:, :, 0:512])
            getattr(nc, q).dma_start(out=x_sb[:, :, 512:XC], in_=xT[nch][:, :, 512:XC])
            for j in range(XC // 512):
                for mi in range(3):
                    ps = qps.tile([P, 512], FP32, name="qkv_acc")
                    for kt in range(C // P):
                        nc.tensor.matmul(
                            ps,
                            w_qkv_sb[:, kt, mi * P : (mi + 1) * P],
                            x_sb[:, kt, j * 512 : (j + 1) * 512],
                            start=(kt == 0),
                            stop=(kt == C // P - 1),
                        )
                    _qkv_evict(
                        qkvT[mi][
                            :, nch * XC + j * 512 : nch * XC + (j + 1) * 512
                        ],
                        ps, mi, j,
                    )

        def emit_v(b, vtp):
            # ones column FIRST: the AV output then has the softmax
            # denominator at PSUM row 0, where reciprocal+partition_broadcast
            # are legal (both are partition-0-only operations).
            nc.vector.memset(v_aug[b][:, :, :, D : D + 1], 1.0)
            for kt in range(KT):
                tok0 = b * T + kt * P
                ps_t = vtp.tile([P, P], BF16, name="vt_t")
                nc.tensor.transpose(ps_t, vT[:, tok0 : tok0 + P], ident)
                nc.vector.tensor_copy(
                    out=v_aug[b][:, kt, :, 0:D],
                    in_=ps_t.rearrange("p (h d) -> p h d", h=HPC),
                )

        def emit_s(b, sps, wlo=0, whi=None, emitted_mask=None, nq=NQ):
            # S^T / exp over the PACKED column space of the P^T store: the
            # causal strips are contiguous, so exp runs in uniform
            # [128, nq] windows instead of ragged per-k-tile chunks.
            # NOTE: Tile orders reads only against writes already emitted, so
            # a window may only be emitted after the qkv chunks covering its
            # q-columns (window 0 is the only chunk-0-pure one for b=0).
            NQ_ = nq
            if whi is None:
                whi = PT_COLS // NQ_
            if emitted_mask is None:
                emitted_mask = set()
            for w in range(wlo, whi):
                w0, w1 = w * NQ_, (w + 1) * NQ_
                ps_s = [sps.tile([P, NQ_], FP32, name=f"s_acc_{h}") for h in range(HPC)]
                for kt in range(KT):
                    a = max(w0, PT_OFF[kt])
                    bnd = min(w1, PT_OFF[kt] + (T - P * kt))
                    if a >= bnd:
                        continue
                    ktok = b * T + kt * P
                    # split at PSUM bank (512) boundaries within the window
                    c = a
                    while c < bnd:
                        nxt = min(bnd, w0 + ((c - w0) // 512 + 1) * 512)
                        q0 = kt * P + (c - PT_OFF[kt])
                        for h in range(HPC):
                            rows = slice(h * D, (h + 1) * D)
                            nc.tensor.matmul(
                                ps_s[h][:, c - w0 : nxt - w0],
                                kTt[rows, ktok : ktok + P],
                                qT[rows, b * T + q0 : b * T + q0 + nxt - c],
                                start=True,
                                stop=True,
                            )
                        c = nxt
                for h in range(HPC):
                    nc.scalar.activation(
                        out=pt_sb[h][:, w0:w1],
                        in_=ps_s[h],
                        func=mybir.ActivationFunctionType.Exp,
                        scale=1.0 / np.sqrt(D),
                    )
                # causal mask for each diagonal block, fired with the
                # window that CONTAINS it (windows may be emitted out of
                # order when interleaved with b0's AV chunks)
                for kt in range(KT):
                    if kt in emitted_mask or PT_OFF[kt] < w0 or PT_OFF[kt] + P > w1:
                        continue
                    emitted_mask.add(kt)
                    for h in range(HPC):
                        nc.vector.tensor_tensor(
                            out=pt_sb[h][:, PT_OFF[kt] : PT_OFF[kt] + P],
                            in0=pt_sb[h][:, PT_OFF[kt] : PT_OFF[kt] + P],
                            in1=tri_sb,
                            op=mybir.AluOpType.mult,
                        )

        def _copy(eng, out, in_):
            # ScalarE copies are ACTIVATE-Copy; DVE copies are TensorCopy.
            if eng == "scalar":
                nc.scalar.copy(out, in_)
            else:
                nc.vector.tensor_copy(out=out, in_=in_)

        def emit_tail(gq, o_engines=("vector", "vector"), oq="sync"):
            """Projection chunk for q-tile gq from normalized yT_sb: both
            512-col matmuls land in one 2-bank PSUM tile, one fp16 evict,
            one store DMA."""
            ci, off = GQ_MAP[gq]
            yt = yT_sb[ci][:, off : off + P]
            ps_o = ops.tile([P, C], FP32, name="o_acc")
            for fj in range(C // 512):
                nc.tensor.matmul(
                    ps_o[:, fj * 512 : (fj + 1) * 512],
                    yt,
                    w_p_sb[:, fj * 512 : (fj + 1) * 512],
                    start=True,
                    stop=True,
                )
            o_sb = osb.tile([P, C], FP16, name="o_stage")
            _copy(o_engines[0], o_sb[:, 0:512], ps_o[:, 0:512])
            _copy(o_engines[1], o_sb[:, 512:C], ps_o[:, 512:C])
            getattr(nc, oq).dma_start(out=out_p[gq * P : (gq + 1) * P, :], in_=o_sb)

        def emit_av_qc(b, t0, t1, avp, paired=True):
            # AV in y^T orientation with row-group pairing: the k-contraction
            # (128 per tile) splits into partitions 0-63 / 64-127; the two
            # groups' matmuls co-execute on the PE (like the S pairs), each
            # accumulating into its own PSUM bank, summed at eviction.
            # Both heads output at PSUM partitions 0-64 (denom row 64);
            # head B's rows are lane-shifted into yT 64-127 by a small
            # SB->SB DMA after normalization (matmul outputs may only start
            # at partition 0/32/64, so 63-offset tricks are illegal).
            q0, q1 = t0, t1
            W_ = t1 - t0
            kmax = q1 // P - 1
            ngrp = 2 if paired else 1
            ps = [
                [avp.tile([D + 1, W_], FP32, name=f"yta_{h}_{rg}") for rg in range(ngrp)]
                for h in range(HPC)
            ]
            for h in range(HPC):
                for kt in range(kmax + 1):
                    sub0 = max(q0, kt * P)
                    col0 = PT_OFF[kt] + sub0 - kt * P
                    if paired:
                        for rg in range(2):
                            nc.tensor.matmul(
                                ps[h][rg][:, sub0 - q0 : W_],
                                v_aug[b][rg * D : (rg + 1) * D, kt, h, :],
                                pt_sb[h][rg * D : (rg + 1) * D, col0 : col0 + q1 - sub0],
                                start=(kt == 0),
                                stop=(kt == kmax),
                            )
                    else:
                        nc.tensor.matmul(
                            ps[h][0][:, sub0 - q0 : W_],
                            v_aug[b][:, kt, h, :],
                            pt_sb[h][:, col0 : col0 + q1 - sub0],
                            start=(kt == 0),
                            stop=(kt == kmax),
                        )
            return ps

        def emit_norm_qc(ci, ps, evict_eng="vector"):
            # stage = ps_rg0+ps_rg1 (rows 0-63 = y dims, row 64 = softmax
            # denominator). reciprocal_approx_fast on the denom row (DVE),
            # tiny DMA moves it to partition 0 (partition_broadcast is
            # strictly lane0 -> lane0+), one pbcast (GpSimd, the ONLY
            # gpsimd op family in steady state -- anything else forces a
            # ~6us microcode library swap), then DVE muls: head A lands in
            # yT_sb directly, head B via an SB->SB DMA lane shift.
            b, t0, t1 = CHUNKS[ci]
            W_ = t1 - t0
            yt = yT_sb[ci]
            stage = stage_pool.tile([D + 1, 2 * W_], FP32, name="y_stage")
            for h in range(HPC):
                cs = slice(h * W_, (h + 1) * W_)
                if len(ps[h]) == 2:
                    # TensorTensor may read only one PSUM operand: ScalarE
                    # evicts row-group 1 to SBUF fp32, DVE adds the other.
                    rg1 = avsb_pool.tile([D + 1, QQ], FP32, name="av_rg1")
                    nc.scalar.copy(rg1, ps[h][1])
                    nc.vector.tensor_tensor(
                        out=stage[:, cs], in0=ps[h][0], in1=rg1,
                        op=mybir.AluOpType.add,
                    )
                else:
                    e = evict_eng if isinstance(evict_eng, str) else evict_eng[h]
                    if e == "scalar":
                        nc.scalar.copy(stage[:, cs], ps[h][0])
                    else:
                        nc.vector.tensor_copy(out=stage[:, cs], in_=ps[h][0])
            # custom-DVE ops (approx reciprocal) and partition_broadcast
            # both require partition-0 operands: shift the denom row down
            # first (SB->SB DMA is partition-free), then recip, then bcast.
            den0 = bsrc_pool.tile([1, 2 * W_], FP32, name="den0")
            nc.gpsimd.dma_start(out=den0, in_=stage[D : D + 1, :])
            rcp = rcp_pool.tile([1, 2 * W_], FP32, name="rcp_row")
            nc.vector.reciprocal_approx_fast(out=rcp, in_=den0)
            bc = bc_pool.tile([D, 2 * W_], FP32, name="bcast")
            nc.gpsimd.partition_broadcast(bc, rcp)
            nc.vector.tensor_tensor(
                out=yt[0:D, :], in0=stage[0:D, 0:W_], in1=bc[:, 0:W_],
                op=mybir.AluOpType.mult,
            )
            tmp = tmpb_pool.tile([D, W_], BF16, name="ytmp")
            nc.vector.tensor_tensor(
                out=tmp, in0=stage[0:D, W_ : 2 * W_], in1=bc[:, W_ : 2 * W_],
                op=mybir.AluOpType.mult,
            )
            nc.gpsimd.dma_start(out=yt[D:P, :], in_=tmp)

        # b=0 tokens live in x chunks 0..T//XC-1
        emit_qkv0()
        qps0.release()
        # s_ps_0 sits at the bottom of the PSUM stack so qkv/vt free their
        # banks for the AV pools while S(b0) is live.
        sps0 = tc.alloc_tile_pool(name="s_ps_0", bufs=1, space="PSUM")
        qps = tc.alloc_tile_pool(name="qkv_ps", bufs=2, space="PSUM")
        # S(b0) window 0 only needs chunk 0, so the exp stream starts early;
        # the remaining windows follow qkv(1) (their q-columns span chunk 1
        # -- Tile cannot order reads against later-emitted writes).
        mask0 = set()
        emit_s(0, sps0, 0, 1, mask0)
        # chunks 1-3 ride sync behind w+bias (gpsimd queue stays clear
        # for chunk 0's pieces; a second stream would starve them).
        emit_qkv(1)
        nc.sync.dma_start(out=w_p_sb, in_=w_p)
        with tc.tile_pool(name="vt_ps", bufs=2, space="PSUM") as vtp:
            emit_v(0, vtp)
            emit_s(0, sps0, 1, PT_COLS // NQ, mask0)
            for nch in range(T // XC, TOK // XC):
                emit_qkv(nch)
            emit_v(1, vtp)
        qps.release()
        xp.release()
        sps0.release()
        # o_ps sits at the BOTTOM of the PSUM stack (banks 0-3) so the
        # projection tiles never WAR-chase the AV pools above them; av/S
        # pools rotate through banks 4-7.
        NB0 = T // QQ                 # b0 chunk count (4)
        NB1 = len(CHUNKS) - NB0       # b1 chunk count (5; last two 256-wide)

        def tails_of(ci, o_engines=None, oqs=None):
            b, t0, t1 = CHUNKS[ci]
            for qt in range((t1 - t0) // P):
                gq = b * KT + t0 // P + qt
                if o_engines is None:
                    # 3:1 ScalarE:DVE -- DVE paces the b1 drain region
                    # while ScalarE idles post-exp
                    oe = ("scalar", "vector") if qt % 2 == 0 else ("scalar", "scalar")
                else:
                    oe = o_engines
                emit_tail(gq, o_engines=oe,
                          oq="sync" if oqs is None else oqs[qt % len(oqs)])

        # Window w of S(b1) overwrites pt columns that only SOME b0 AV
        # chunks read: group the windows by the LAST b0 chunk whose reads
        # they must wait for, and emit each group right after that chunk.
        # exp(b1) then streams on ScalarE ~30us earlier, overlapped with
        # the b0 chains/tails, instead of serializing after all of AV(b0).
        NQ1 = 512

        def win_maxqc(w):
            w0, w1 = w * NQ1, (w + 1) * NQ1
            mq = 0
            for kt in range(KT):
                a = max(w0, PT_OFF[kt])
                bnd = min(w1, PT_OFF[kt] + (T - P * kt))
                if a >= bnd:
                    continue
                mq = max(mq, ((bnd - 1) - PT_OFF[kt] + P * kt) // QQ)
            return mq

        WGRP = {}
        for w in range(PT_COLS // NQ1):
            WGRP.setdefault(win_maxqc(w), []).append(w)

        with tc.tile_pool(name="o_ps", bufs=2, space="PSUM") as ops:
            with tc.tile_pool(name="s_ps_1", bufs=1, space="PSUM") as sps1, \
                 tc.tile_pool(name="av_ps_0", bufs=1, space="PSUM") as avp:
                m1 = set()
                prev = None
                for ci in range(NB0):
                    _, t0, t1 = CHUNKS[ci]
                    ps = emit_av_qc(0, t0, t1, avp, paired=False)
                    # S(b1) windows unlocked by this chunk double as PE
                    # filler across the bufs=1 WAR seam
                    for w in WGRP.get(ci, []):
                        emit_s(1, sps1, w, w + 1, m1, nq=NQ1)
                    if prev is not None:
                        # exp(b1) owns ScalarE here: b0 tails stay on DVE,
                        # chain stage-evicts ride ScalarE between windows
                        emit_norm_qc(ci - 1, prev, evict_eng="scalar")
                        tails_of(ci - 1, o_engines=("vector", "vector"))
                    prev = ps
                emit_norm_qc(NB0 - 1, prev, evict_eng="scalar")
                tails_of(NB0 - 1, o_engines=("vector", "vector"))
            # b1 AV rides right behind exp(b1), software-pipelined one
            # stage; the final two short chunks shorten the drain and the
            # last stores fan out over the three DMA queues.
            with tc.tile_pool(name="av_ps_1", bufs=2, space="PSUM") as avp1:
                prev = None
                for k in range(NB1):
                    ci = NB0 + k
                    _, t0, t1 = CHUNKS[ci]
                    ps = emit_av_qc(1, t0, t1, avp1, paired=False)
                    if prev is not None:
                        emit_norm_qc(ci - 1, prev,
                                     evict_eng="scalar" if k >= NB1 - 1 else "vector")
                        tails_of(ci - 1)
                    prev = ps
                emit_norm_qc(NB0 + NB1 - 1, prev, evict_eng="scalar")
                tails_of(NB0 + NB1 - 1, oqs=("scalar", "gpsimd"))


def shard_inputs(x, W_attn, b_attn, W_proj, b_proj):
    x = np.asarray(x, np.float32)
    W_attn = np.asarray(W_attn, np.float32)
    b_attn = np.asarray(b_attn, np.float32)
    W_proj = np.asarray(W_proj, np.float32)

    # [chunk, p, ktile, tok]: contiguous per-chunk DMA source for x^T
    xT = np.ascontiguousarray(
        x.reshape(TOK // XC, XC, C // P, P).transpose(0, 3, 2, 1)
    ).astype(bf16)
    # causal keep-mask for diagonal blocks: keep P^T[k, q] where q >= k
    tri_mask = np.triu(np.ones((P, P), np.float32)).astype(bf16)
    in_maps = []
    for c in range(N_CORES):
        fs = slice(P * c, P * (c + 1))
        w_slice = np.ascontiguousarray(
            np.concatenate(
                [W_attn[:, 0 * C + P * c : 0 * C + P * (c + 1)],
                 W_attn[:, 1 * C + P * c : 1 * C + P * (c + 1)],
                 W_attn[:, 2 * C + P * c : 2 * C + P * (c + 1)]],
                axis=1,
            )
        ).astype(bf16)
        # [128, 3] host-prepack: column m = bias of feature m*128+p, so the
        # on-core DMA is 128 contiguous 12B lines instead of 384 4B gathers.
        b_slice = np.ascontiguousarray(
            np.stack([b_attn[0 * C + P * c : 0 * C + P * (c + 1)],
                      b_attn[1 * C + P * c : 1 * C + P * (c + 1)],
                      b_attn[2 * C + P * c : 2 * C + P * (c + 1)]], axis=1)
        ).astype(np.float32)
        wp_slice = np.ascontiguousarray(W_proj[fs, :]).astype(bf16)
        in_maps.append(
            {"xT": xT, "w_qkv": w_slice, "b_qkv": b_slice, "w_p": wp_slice,
             "tri": tri_mask}
        )
    return in_maps


def kernel(x, W_attn, b_attn, W_proj, b_proj, _trace=False):
    in_maps = shard_inputs(x, W_attn, b_attn, W_proj, b_proj)
    nc = build_nc()
    res = run_bass_kernel_spmd(nc, in_maps, list(range(N_CORES)), trace=_trace)
    acc = np.zeros((TOK, C), np.float64)
    for r in res.results:
        acc += r["out_p"].astype(np.float64)
    out = acc.astype(np.float32) + np.asarray(b_proj, np.float32)[None, :]
    if _trace:
        kernel.last_results = res
    return out.reshape(B, T, C)
